# revision 4
# baseline (speedup 1.0000x reference)
"""GCN (2x GCNConv + mean-pool + fc + LayerNorm) on 8 Trainium2 NeuronCores.

Strategy: shard nodes (and their in-edges) across 8 cores. Each GCNConv is
  gather x[src] (gpsimd dma_gather) -> scale by dinv[src]*dinv[dst] (DVE)
  -> duplicate-free scatter-add batches into z[dst] (gpsimd dma_scatter_add)
  -> PE transpose + matmul (z @ W + b) -> relu.
AllGather replicates h between the convs; pooling is a PSUM-accumulated
matmul with an on-device one-hot graph-assignment matrix; pooled sums are
AllReduced; the tiny fc+LayerNorm head is computed redundantly on all cores.
"""
import sys

if '/opt/trn_rl_repo' not in sys.path:
    sys.path.insert(0, '/opt/trn_rl_repo')

import ml_dtypes
FP8 = None  # set below
import numpy as np

import concourse.bacc as bacc
import concourse.mybir as mybir
from concourse.tile import TileContext
from concourse.bass_utils import run_bass_kernel_spmd

# ---------------------------------------------------------------- constants
N = 100000
E = 800000
IN = 64
HID = 128
G = 256
NC = 8
RPC = N // NC              # 12500 rows (nodes) per core
NCHUNK = (RPC + 127) // 128        # 98
RPC_PAD = NCHUNK * 128             # 12544
TRASH = 2048
U_ROWS = 2 * RPC + TRASH           # scatter target rows (dst*2+slot | trash)
LN_EPS = 1e-5
TCH1 = 96                  # conv1 tile: chunks of 128 edges
TCH2 = 64                  # conv2 tile
GRP1 = 7                   # mm1 chunks per DMA group (divides 98)
GRP2 = 7                   # mm2 chunks per DMA group
NSLICE = 1                 # sliced AllGather loses: collective cost is fixed-overhead dominated
SLICE_ROWS = RPC_PAD // NSLICE
F32 = mybir.dt.float32
BF16 = mybir.dt.bfloat16
F8 = mybir.dt.float8e4
I16 = mybir.dt.int16


def _wrap16(a, cols):
    """[n] -> [128, cols] int16: element i -> [i%16, i//16], tiled x8."""
    out = np.zeros((16, cols), np.int16)
    w = a.reshape(-1, 16).T
    out[:, : w.shape[1]] = w
    return np.tile(out, (8, 1))


def _host_prep(edge_index, batch):
    """Build per-core padded edge streams + common tile structure."""
    src = np.asarray(edge_index[0], np.int64)
    dst = np.asarray(edge_index[1], np.int64)
    deg = (np.bincount(dst, minlength=N) + 1.0).astype(np.float32)
    dinv = (1.0 / np.sqrt(deg)).astype(np.float32)
    coef_all = (dinv[src] * dinv[dst]).astype(np.float32)

    cores = []
    for c in range(NC):
        m = (dst >= c * RPC) & (dst < (c + 1) * RPC)
        sl = np.arange(RPC, dtype=np.int64)
        s = np.concatenate([sl + c * RPC, src[m]])
        d = np.concatenate([sl, dst[m] - c * RPC])
        cf = np.concatenate([dinv[c * RPC:(c + 1) * RPC] ** 2, coef_all[m]])
        # rank within dst; stable sort puts the self-edge (listed first) at rank 0
        order = np.argsort(d, kind='stable')
        ds = d[order]
        starts = np.r_[0, np.flatnonzero(np.diff(ds)) + 1]
        seg_len = np.diff(np.r_[starts, len(ds)])
        rk_sorted = np.arange(len(ds)) - np.repeat(starts, seg_len)
        rank = np.empty(len(ds), np.int64)
        rank[order] = rk_sorted
        cores.append((s, d, cf, rank))

    sb_count = max(int(r.max()) for _, _, _, r in cores) // 2 + 1

    # common segment sizes SEG[sb][g8] (multiples of 128; max over cores).
    # g8 = (src%4)*2 + is_remote: "local" edges (src in this core's range)
    # gather from hpart and can overlap the AllGather. The interleaved order
    # lets conv1 merge each local/remote pair into one gather call.
    NG = 8
    seg = np.zeros((sb_count, NG), np.int64)
    percore_segs = []
    for ci, (s, d, cf, rank) in enumerate(cores):
        sb = rank // 2
        g = (s % 4) * 2 + (s // RPC != ci).astype(np.int64)
        cnt = np.zeros((sb_count, NG), np.int64)
        np.add.at(cnt, (sb, g), 1)
        percore_segs.append(cnt)
        seg = np.maximum(seg, cnt)
    seg = (seg + 127) // 128 * 128

    span = seg.sum(axis=1)              # edges per super-batch (mult of 128)
    sb_off = np.r_[0, np.cumsum(span)]
    epad = int(sb_off[-1])
    ecols = epad // 16

    # static tile/call structures (identical for every core); each tile is
    # one gather + one duplicate-free scatter. conv1 merges local+remote
    # pairs (4 groups); conv2 keeps them separate (8 groups, local first).
    tiles1 = []                    # (chunk_lo, chunk_hi, parity, g4)
    tiles2 = []                    # (chunk_lo, chunk_hi, parity, g8)
    for b in range(sb_count):
        base = int(sb_off[b]) // 128
        g_edges = [(int(x) // 128) for x in seg[b]]
        g_bounds = np.r_[0, np.cumsum(g_edges)]           # in chunks, rel
        for g4 in range(4):
            lo = int(g_bounds[2 * g4])
            end = int(g_bounds[2 * g4 + 2])
            while lo < end:
                hi = min(lo + TCH1, end)
                tiles1.append((base + lo, base + hi, len(tiles1) % 2, g4))
                lo = hi
        for g in range(NG):
            lo = int(g_bounds[g])
            while lo < int(g_bounds[g + 1]):
                hi = min(lo + TCH1, int(g_bounds[g + 1]))
                tiles2.append((base + lo, base + hi, len(tiles2) % 2, g))
                lo = hi

    # per-core streams
    per_core = []
    for ci, (s, d, cf, rank) in enumerate(cores):
        sb = rank // 2
        slot = rank % 2
        g = (s % 4) * 2 + (s // RPC != ci).astype(np.int64)
        key = sb * NG + g
        order = np.argsort(key, kind='stable')
        s, d, cf, sb, slot, g = (x[order] for x in (s, d, cf, sb, slot, g))
        # position of each edge in the padded stream
        cnt = percore_segs[ci]
        gi1 = np.zeros(epad, np.int16)
        gi2 = np.zeros(epad, np.int16)
        si = np.empty(epad, np.int64)
        si[:] = 2 * RPC + (np.arange(epad) % TRASH)       # default: trash
        cfp = np.zeros(epad, np.float32)
        ptr = 0
        for b in range(sb_count):
            for gg in range(NG):
                n = int(cnt[b][gg])
                off = int(sb_off[b]) + int(seg[b][:gg].sum())
                sl = slice(ptr, ptr + n)
                gi1[off:off + n] = (s[sl] // 4).astype(np.int16)
                loc = s[sl] % RPC
                if gg % 2 == 0:   # local: row in this core's hpart
                    hrow = loc
                else:             # remote: row in hfull (core-major, padded)
                    hrow = (s[sl] // RPC) * RPC_PAD + loc
                gi2[off:off + n] = (hrow // 4).astype(np.int16)
                si[off:off + n] = 2 * d[sl] + slot[sl]
                cfp[off:off + n] = cf[sl]
                ptr += n
        per_core.append({
            "gi1": _wrap16(gi1, ecols),
            "gi2": _wrap16(gi2, ecols),
            "si": _wrap16(si.astype(np.int16), ecols),
            "cf": cfp.reshape(-1, 128).T.copy(),          # [128, epad/128]
        })

    # per-node graph ids (pad chunks -> -1), per-core [128, NCHUNK] f32
    gid = np.asarray(batch, np.int64)
    for ci in range(NC):
        gv = np.full(RPC_PAD, -1.0, np.float32)
        gv[:RPC] = gid[ci * RPC:(ci + 1) * RPC].astype(np.float32)
        per_core[ci]["gid"] = gv.reshape(NCHUNK, 128).T.copy()   # [128, NCHUNK]

    cntg = np.bincount(gid, minlength=G).astype(np.float32)
    inv_cnt = (1.0 / np.maximum(cntg, 1.0)).astype(np.float32)
    inv_cnt_w = inv_cnt.reshape(2, 128).T.copy()                  # [128, 2]

    meta = {"sb_count": sb_count, "epad": epad, "ecols": ecols,
            "tiles": tiles1, "tiles2": tiles2}
    return per_core, inv_cnt_w, meta


def _build(meta, stage=5):
    """Build + compile the 8-core Bass kernel for the given edge structure.

    stage: 1=conv1 scatter, 2=+conv1 matmul, 3=+AllGather, 4=+conv2 scatter,
    5=full. Stages <5 write debug tensors and stop.
    """
    nc = bacc.Bacc("TRN2", target_bir_lowering=False, debug=False,
                   num_devices=NC, num_swdge_queues=4)
    epad, ecols = meta["epad"], meta["ecols"]
    tiles1, tiles2 = meta["tiles"], meta["tiles2"]

    # ------------------------------------------------ I/O declarations
    x_d = nc.dram_tensor("x", [N, IN], F32, kind="ExternalInput")
    w1_d = nc.dram_tensor("w1", [IN, HID], F32, kind="ExternalInput")
    w2_d = nc.dram_tensor("w2", [HID, HID], F32, kind="ExternalInput")
    wfc_d = nc.dram_tensor("wfc", [HID, HID], F32, kind="ExternalInput")
    b1r_d = nc.dram_tensor("b1r", [128, HID], F32, kind="ExternalInput")
    b2r_d = nc.dram_tensor("b2r", [128, HID], F32, kind="ExternalInput")
    bfcr_d = nc.dram_tensor("bfcr", [128, HID], F32, kind="ExternalInput")
    gamr_d = nc.dram_tensor("gamr", [128, HID], F32, kind="ExternalInput")
    betr_d = nc.dram_tensor("betr", [128, HID], F32, kind="ExternalInput")
    gi1_d = nc.dram_tensor("gi1", [128, ecols], I16, kind="ExternalInput")
    gi2_d = nc.dram_tensor("gi2", [128, ecols], I16, kind="ExternalInput")
    si_d = nc.dram_tensor("si", [128, ecols], I16, kind="ExternalInput")
    cf_d = nc.dram_tensor("cf", [128, epad // 128], F32, kind="ExternalInput")
    gid_d = nc.dram_tensor("gid", [128, NCHUNK], F32, kind="ExternalInput")
    icnt_d = nc.dram_tensor("icnt", [128, 2], F32, kind="ExternalInput")
    u1 = [nc.dram_tensor(f"u1{p}", [U_ROWS, 4 * IN], F8, kind="ExternalInput")
          for p in range(2)]
    u2 = [nc.dram_tensor(f"u2{p}", [U_ROWS, 2 * HID], F8, kind="ExternalInput")
          for p in range(2)]
    y_d = nc.dram_tensor("y", [G, HID], F32, kind="ExternalOutput")
    if stage == 1:
        dbg_u = [nc.dram_tensor(f"dbg_u{p}", [U_ROWS, IN], F32,
                                kind="ExternalOutput") for p in range(2)]
    if stage == 2 or stage == 3:
        dbg_h = nc.dram_tensor("dbg_h", [RPC_PAD, HID], BF16,
                               kind="ExternalOutput")
    if stage == 4:
        dbg_u = [nc.dram_tensor(f"dbg_v{p}", [U_ROWS, HID], BF16,
                                kind="ExternalOutput") for p in range(2)]

    eye_d = nc.inline_tensor(np.eye(128, dtype=np.float32), name="eye128")
    iota_d = nc.inline_tensor(
        np.tile(np.arange(256, dtype=np.float32), (128, 1)), name="iota256")

    hpart = nc.dram_tensor("hpart", [RPC_PAD, HID], BF16)
    hfull = nc.dram_tensor("hfull", [NC * RPC_PAD, HID], BF16,
                           addr_space="Shared")
    pool_loc = nc.dram_tensor("pool_loc", [G, HID], F32)
    pool_glob = nc.dram_tensor("pool_glob", [G, HID], F32, addr_space="Shared")

    x4 = x_d.ap().rearrange("(a b) d -> a b d", b=4)          # [25000,4,64]
    h4 = hfull.ap().rearrange("(a b) d -> a b d", b=4)        # remote table
    hp4 = hpart.ap().rearrange("(a b) d -> a b d", b=4)       # local table

    # persistent SBUF (index/coef streams + small constants)
    gi1_s = nc.alloc_sbuf_tensor("gi1_s", [128, ecols], I16)
    gi2_s = nc.alloc_sbuf_tensor("gi2_s", [128, ecols], I16)
    si_s = nc.alloc_sbuf_tensor("si_s", [128, ecols], I16)
    cf_s = nc.alloc_sbuf_tensor("cf_s", [128, epad // 128], F32)

    CORES = [list(range(NC))]

    def conv_scatter(tc, pool, conv):
        """gather -> coef-scale -> duplicate-free scatter-add batches."""
        D = IN if conv == 1 else HID
        gidx = gi1_s if conv == 1 else gi2_s
        ustep = 4 * D
        u = u1 if conv == 1 else u2
        sstep = (4 * D) if conv == 1 else (2 * D)
        uap = (lambda p: u[p].ap()[:, 0:D])
        tmax = TCH1 if conv == 1 else TCH2
        qn = 0
        gdt = F32 if conv == 1 else BF16
        sdt = F8
        # conv2: local-gather tiles (g8 even) first so they overlap AllGather
        order = tiles1 if conv == 1 else sorted(tiles2, key=lambda t: t[3] % 2)
        for (lo, hi, parity, g8) in order:
            if conv == 1:
                table, grp = x4, g8
            else:
                table, grp = (hp4 if g8 % 2 == 0 else h4), g8 // 2
            # conv2 tiles were sized for TCH1; split to TCH2 granularity
            sub = []
            a = lo
            while a < hi:
                b = min(a + tmax, hi)
                sub.append((a, b))
                a = b
            for (slo, shi) in sub:
                nchk = shi - slo
                n = nchk * 128
                t = pool.tile([128, nchk, D], gdt, tag=f"gt{conv}")
                nc.gpsimd.dma_gather(
                    t[:], table[:, grp, :], gidx[:, slo * 8:shi * 8],
                    n, n, D, elem_step=ustep, queue_num=qn % 4,
                    single_packet=False,
                )
                tf = pool.tile([128, nchk, D], sdt, tag="gtf")
                nc.vector.tensor_tensor(
                    tf[:],
                    t[:],
                    cf_s.ap()[:, slo:shi]
                    .rearrange("p (c o) -> p c o", o=1)
                    .broadcast_to([128, nchk, D]),
                    mybir.AluOpType.mult,
                )
                nc.gpsimd.dma_scatter_add(
                    uap(parity), tf[:], si_s[:, slo * 8:shi * 8], n, n, D,
                    elem_step=sstep, queue_num=qn % 4, single_packet=False,
                )
                qn += 1

    try:
      with TileContext(nc) as tc:
        with tc.tile_pool(name="init", bufs=1) as ipool:
            nc.sync.dma_start(out=gi1_s[:], in_=gi1_d[:])
            nc.sync.dma_start(out=gi2_s[:], in_=gi2_d[:])
            nc.sync.dma_start(out=si_s[:], in_=si_d[:])
            nc.sync.dma_start(out=cf_s[:], in_=cf_d[:])
            eye_t = ipool.tile([128, 128], F32)
            nc.sync.dma_start(out=eye_t[:], in_=eye_d[:])
            w1_t = ipool.tile([IN, HID], F32)
            nc.sync.dma_start(out=w1_t[:], in_=w1_d[:])
            w2_t = ipool.tile([HID, HID], F32)
            nc.sync.dma_start(out=w2_t[:], in_=w2_d[:])
            b1r_t = ipool.tile([128, HID], F32)
            nc.sync.dma_start(out=b1r_t[:], in_=b1r_d[:])
            b2r_t = ipool.tile([128, HID], F32)
            nc.sync.dma_start(out=b2r_t[:], in_=b2r_d[:])
            gid_t = ipool.tile([128, NCHUNK], F32)
            nc.sync.dma_start(out=gid_t[:], in_=gid_d[:])
            iota_t = ipool.tile([128, 256], F32)
            nc.sync.dma_start(out=iota_t[:], in_=iota_d[:])

            # ---------------- conv1: edge aggregation into u1[0|1]
            with tc.tile_pool(name="sc1", bufs=4) as spool:
                conv_scatter(tc, spool, 1)
            if stage == 1:
                for p in range(2):
                    nc.sync.dma_start(out=dbg_u[p][:], in_=u1[p][:])

            # ---------------- conv1: h = relu(z @ W1 + b1) ; h -> hpart
            GRP = GRP1
            u1r = [u.ap()[:NCHUNK * 256, :]
                   .rearrange("(a p s) f -> p a s f", p=128, s=2)
                   for u in u1]
            hp_r = hpart.ap().rearrange("(a p) f -> p a f", p=128)
            with (
                tc.tile_pool(name="mm1", bufs=3) as mpool,
                tc.tile_pool(name="ps1", bufs=3, space="PSUM") as ppool,
            ):
                for a0 in range(0, NCHUNK, GRP):
                    ga = mpool.tile([128, GRP, 2, 4 * IN], F8, tag="ua")
                    gb = mpool.tile([128, GRP, 2, 4 * IN], F8, tag="ub")
                    nc.sync.dma_start(out=ga[:], in_=u1r[0][:, a0:a0 + GRP])
                    nc.sync.dma_start(out=gb[:], in_=u1r[1][:, a0:a0 + GRP])
                    z = mpool.tile([128, GRP, IN], F32, tag="z")
                    nc.gpsimd.tensor_add(z[:], ga[:, :, 0, 0:IN], ga[:, :, 1, 0:IN])
                    nc.gpsimd.tensor_add(z[:], z[:], gb[:, :, 0, 0:IN])
                    nc.vector.tensor_add(z[:], z[:], gb[:, :, 1, 0:IN])
                    hg = mpool.tile([128, GRP, HID], BF16, tag="hg")
                    for j in range(GRP):
                        zT_p = ppool.tile([IN, 128], F32, tag="zT")
                        nc.tensor.transpose(zT_p[:], z[:, j, :], eye_t[:])
                        zT_s = mpool.tile([IN, 128], F32, tag="zTs")
                        nc.vector.tensor_copy(zT_s[:], zT_p[:])
                        h_p = ppool.tile([128, HID], F32, tag="hp")
                        nc.tensor.matmul(h_p[:], zT_s[:], w1_t[:])
                        nc.vector.tensor_add(h_p[:], h_p[:], b1r_t[:])
                        nc.scalar.activation(hg[:, j, :], h_p[:],
                                             mybir.ActivationFunctionType.Relu)
                    nc.sync.dma_start(out=hp_r[:, a0:a0 + GRP], in_=hg[:])

            if stage == 2:
                nc.sync.dma_start(out=dbg_h[:], in_=hpart[:])
            # ---------------- replicate h across cores: one AllGather per
            # slice into separate tensors so conv2's slice-0 gathers can
            # overlap the slice-1 transfer
            if stage >= 3:
                nc.gpsimd.collective_compute(
                    "AllGather", mybir.AluOpType.bypass, CORES,
                    [hpart[:]], [hfull[:]],
                )
            if stage == 3:
                nc.sync.dma_start(out=dbg_h[:],
                                  in_=hfull.ap()[RPC_PAD:2 * RPC_PAD, :])
            # ---------------- conv2: edge aggregation into u2[0|1]
            if stage >= 4:
                with tc.tile_pool(name="sc2", bufs=4) as spool:
                    conv_scatter(tc, spool, 2)
            if stage == 4:
                for p in range(2):
                    nc.sync.dma_start(out=dbg_u[p][:], in_=u2[p][:])

            if stage >= 5:
                # ---------------- conv2 matmul + relu + pooling matmul
                u2r = [u.ap()[:NCHUNK * 256, :]
                       .rearrange("(a p s) f -> p a s f", p=128, s=2)
                       for u in u2]
                with (
                    tc.tile_pool(name="mm2", bufs=4) as mpool,
                    tc.tile_pool(name="ps2", bufs=3, space="PSUM") as ppool,
                    tc.tile_pool(name="pacc", bufs=1, space="PSUM") as accpool,
                ):
                    pooled = [accpool.tile([128, HID], F32, tag=f"pool{h}",
                                           name=f"pooled{h}")
                              for h in range(2)]
                    for a0 in range(0, NCHUNK, GRP2):
                        ga = mpool.tile([128, GRP2, 2, 2 * HID], F8, tag="ua")
                        gb = mpool.tile([128, GRP2, 2, 2 * HID], F8, tag="ub")
                        nc.sync.dma_start(out=ga[:], in_=u2r[0][:, a0:a0 + GRP2])
                        nc.sync.dma_start(out=gb[:], in_=u2r[1][:, a0:a0 + GRP2])
                        z = mpool.tile([128, GRP2, HID], F32, tag="z")
                        nc.gpsimd.tensor_add(z[:], ga[:, :, 0, 0:HID], ga[:, :, 1, 0:HID])
                        nc.gpsimd.tensor_add(z[:], z[:], gb[:, :, 0, 0:HID])
                        nc.vector.tensor_add(z[:], z[:], gb[:, :, 1, 0:HID])
                        for j in range(GRP2):
                            a = a0 + j
                            zT_p = ppool.tile([HID, 128], F32, tag="zT")
                            nc.tensor.transpose(zT_p[:], z[:, j, :], eye_t[:])
                            zT_s = mpool.tile([HID, 128], F32, tag="zTs")
                            nc.vector.tensor_copy(zT_s[:], zT_p[:])
                            h_p = ppool.tile([128, HID], F32, tag="hp")
                            nc.tensor.matmul(h_p[:], zT_s[:], w2_t[:])
                            nc.vector.tensor_add(h_p[:], h_p[:], b2r_t[:])
                            h2_s = mpool.tile([128, HID], F32, tag="h2s")
                            nc.scalar.activation(h2_s[:], h_p[:],
                                                 mybir.ActivationFunctionType.Relu)
                            sel = mpool.tile([128, 256], F32, tag="sel")
                            nc.vector.tensor_tensor(
                                sel[:],
                                gid_t[:, a:a + 1].broadcast_to([128, 256]),
                                iota_t[:],
                                mybir.AluOpType.is_equal,
                            )
                            for hh in range(2):
                                nc.tensor.matmul(
                                    pooled[hh][:],
                                    sel[:, hh * 128:(hh + 1) * 128], h2_s[:],
                                    start=(a == 0), stop=(a == NCHUNK - 1),
                                )
                    # pooled sums -> dram
                    pl_r = pool_loc.ap().rearrange("(h p) f -> p h f", p=128)
                    pl_s = mpool.tile([128, 2, HID], F32, tag="pls")
                    nc.vector.tensor_copy(pl_s[:, 0, :], pooled[0][:])
                    nc.vector.tensor_copy(pl_s[:, 1, :], pooled[1][:])
                    nc.sync.dma_start(out=pl_r[:], in_=pl_s[:])

                nc.gpsimd.collective_compute(
                    "AllReduce", mybir.AluOpType.add, CORES,
                    [pool_loc[:]], [pool_glob[:]],
                )

                # ---------------- head: mean-div, fc, LayerNorm (tiny)
                pg_r = pool_glob.ap().rearrange("(h p) f -> p h f", p=128)
                y_r = y_d.ap().rearrange("(h p) f -> p h f", p=128)
                with (
                    tc.tile_pool(name="head", bufs=1) as hpool,
                    tc.tile_pool(name="psh", bufs=2, space="PSUM") as hps,
                ):
                    wfc_t = hpool.tile([HID, HID], F32)
                    nc.sync.dma_start(out=wfc_t[:], in_=wfc_d[:])
                    bfcr_t = hpool.tile([128, HID], F32)
                    nc.sync.dma_start(out=bfcr_t[:], in_=bfcr_d[:])
                    gamr_t = hpool.tile([128, HID], F32)
                    nc.sync.dma_start(out=gamr_t[:], in_=gamr_d[:])
                    betr_t = hpool.tile([128, HID], F32)
                    nc.sync.dma_start(out=betr_t[:], in_=betr_d[:])
                    icnt_t = hpool.tile([128, 2], F32)
                    nc.sync.dma_start(out=icnt_t[:], in_=icnt_d[:])
                    eps_t = hpool.tile([128, 1], F32)
                    nc.vector.memset(eps_t[:], LN_EPS)
                    yo = hpool.tile([128, 2, HID], F32)
                    for hh in range(2):
                        pg_s = hpool.tile([128, HID], F32, tag="pg")
                        nc.sync.dma_start(out=pg_s[:], in_=pg_r[:, hh, :])
                        nc.vector.tensor_scalar(
                            pg_s[:], pg_s[:], icnt_t[:, hh:hh + 1], None,
                            mybir.AluOpType.mult)
                        pgT_p = hps.tile([HID, 128], F32, tag="pgT")
                        nc.tensor.transpose(pgT_p[:], pg_s[:], eye_t[:])
                        pgT_s = hpool.tile([HID, 128], F32, tag="pgTs")
                        nc.vector.tensor_copy(pgT_s[:], pgT_p[:])
                        y_p = hps.tile([128, HID], F32, tag="yp")
                        nc.tensor.matmul(y_p[:], pgT_s[:], wfc_t[:])
                        y_s = hpool.tile([128, HID], F32, tag="ys")
                        nc.vector.tensor_add(y_s[:], y_p[:], bfcr_t[:])
                        # LayerNorm along features (free dim)
                        mu = hpool.tile([128, 1], F32, tag="mu")
                        nc.vector.tensor_reduce(mu[:], y_s[:],
                                                mybir.AxisListType.XYZW,
                                                mybir.AluOpType.add)
                        nc.vector.tensor_scalar(mu[:], mu[:], -1.0 / HID, None,
                                                mybir.AluOpType.mult)
                        cen = hpool.tile([128, HID], F32, tag="cen")
                        nc.vector.tensor_scalar(cen[:], y_s[:], mu[:], None,
                                                mybir.AluOpType.add)
                        sq = hpool.tile([128, HID], F32, tag="sq")
                        nc.vector.tensor_mul(sq[:], cen[:], cen[:])
                        var = hpool.tile([128, 1], F32, tag="var")
                        nc.vector.tensor_reduce(var[:], sq[:],
                                                mybir.AxisListType.XYZW,
                                                mybir.AluOpType.add)
                        std = hpool.tile([128, 1], F32, tag="std")
                        nc.scalar.activation(std[:], var[:],
                                             mybir.ActivationFunctionType.Sqrt,
                                             bias=eps_t[:], scale=1.0 / HID)
                        rstd = hpool.tile([128, 1], F32, tag="rstd")
                        nc.vector.reciprocal(rstd[:], std[:])
                        nc.vector.tensor_scalar(cen[:], cen[:], rstd[:], None,
                                                mybir.AluOpType.mult)
                        nc.vector.tensor_mul(cen[:], cen[:], gamr_t[:])
                        nc.vector.tensor_add(yo[:, hh, :], cen[:], betr_t[:])
                    nc.sync.dma_start(out=y_r[:], in_=yo[:])

    finally:
        pass
    nc.compile()
    return nc


class _StopBuild(Exception):
    """Truncate the build at a debug stage (unwinds out of TileContext)."""


_CACHE = {}


def kernel(x, edge_index, batch, W1, b1, W2, b2, Wfc, bfc, gamma, beta,
           _stage=5, _full_results=False):
    x = np.asarray(x, np.float32)
    per_core, inv_cnt_w, meta = _host_prep(np.asarray(edge_index),
                                           np.asarray(batch))
    key = (meta["epad"], meta["sb_count"], len(meta["tiles"]), _stage)
    if key not in _CACHE:
        _CACHE[key] = _build(meta, _stage)
    nc = _CACHE[key]

    rep = lambda v: np.tile(np.asarray(v, np.float32)[None, :], (128, 1))
    shared = {
        "x": x,
        "w1": np.asarray(W1, np.float32),
        "w2": np.asarray(W2, np.float32),
        "wfc": np.asarray(Wfc, np.float32),
        "b1r": rep(b1), "b2r": rep(b2), "bfcr": rep(bfc),
        "gamr": rep(gamma), "betr": rep(beta),
        "icnt": inv_cnt_w,
        "u10": np.zeros((U_ROWS, 4 * IN), ml_dtypes.float8_e4m3fn),
        "u11": np.zeros((U_ROWS, 4 * IN), ml_dtypes.float8_e4m3fn),
        "u20": np.zeros((U_ROWS, 2 * HID), ml_dtypes.float8_e4m3fn),
        "u21": np.zeros((U_ROWS, 2 * HID), ml_dtypes.float8_e4m3fn),
    }
    in_maps = []
    for c in range(NC):
        m = dict(shared)
        m["gi1"] = per_core[c]["gi1"]
        m["gi2"] = per_core[c]["gi2"]
        m["si"] = per_core[c]["si"]
        m["cf"] = per_core[c]["cf"]
        m["gid"] = per_core[c]["gid"]
        in_maps.append(m)

    res = run_bass_kernel_spmd(nc, in_maps, list(range(NC)))
    if _full_results:
        return res.results
    return res.results[0]["y"]

